# revision 3
# baseline (speedup 1.0000x reference)
"""Cross-attention kernel for Trainium2, 8-core head-sharded (tensor parallel).

Problem: x[2,2048,1024], context[2,2048,768], Wq[1024,1024], Wkv[768,2048],
Wo[1024,1024]; H=16 heads, Dh=64; out = softmax(q k^T / 8) v @ Wo.

Sharding: 2 heads per core (128 q/k/v columns). Each core computes its
heads' attention output projected through its 128-row slice of Wo,
producing a full-shape partial; host sums the 8 partials.

Per-core dataflow (all matmuls bf16 with fp32 PSUM accumulation):
  qT[128,4096]  = Wq_slice^T @ x^T        (lhsT = Wq_slice, rhs = xT)
  kT[128,4096]  = Wk_slice^T @ ctx^T
  v[4096,130]   = ctx @ Wv_slice, +ones columns (flash denominator trick)
  per (batch b, head h, n-tile of 512 queries):
    S^T[m,n]    = kT_h^T-slice matmuls  (K=64)  -> PSUM [128, 4xn512]
    expST       = Exp(S^T * 0.125)  on ScalarE -> SBUF bf16
    OT[65,512] += v_aug_h^T @ expST  (K=128, accumulate over 16 m-tiles)
                  row 64 = softmax denominator
    OT_norm     = OT[0:64] * bcast(1/denom)   (DVE + gpsimd broadcast)
  out[n,1024]   = OTcomb^T @ Wo_slice  -> partial output, DMA to DRAM
"""

import numpy as np
import ml_dtypes

import concourse.bass as bass
import concourse.mybir as mybir
import concourse.tile as tile
from concourse import bacc
from concourse.bass_utils import run_bass_kernel_spmd

BF16 = mybir.dt.bfloat16
F32 = mybir.dt.float32
NPBF16 = ml_dtypes.bfloat16

B, N, C = 2, 2048, 1024
M = 2048          # context length
CTX = 768
H = 16
DH = 64
NCORES = 8
HEADS_PER_CORE = H // NCORES          # 2
HC = HEADS_PER_CORE * DH              # 128 columns per core
ROWS = B * N                          # 4096 flattened rows
SCALE = DH ** -0.5                    # 0.125

KQ = C // 128                         # 8 k-tiles for q projection
KC = CTX // 128                       # 6 k-tiles for kv projections
NT = 512                              # query-tile width (free dim)
MT = 128                              # m-tile (context block = partitions)
N_NT = N // NT                        # 4 query tiles per batch
N_MT = M // MT                        # 16 m-tiles per batch
MG = 2                                # m-tiles per exp group ([128,1024] ACT calls)

_CACHE = {}


def build_kernel():
    """Build and compile the per-core Bass module (same program, all cores)."""
    nc = bacc.Bacc(None)

    xT_d = nc.dram_tensor("xT", [C, ROWS], BF16, kind="ExternalInput")
    cT_d = nc.dram_tensor("ctxT", [CTX, ROWS], BF16, kind="ExternalInput")
    wq_d = nc.dram_tensor("wq", [C, HC], BF16, kind="ExternalInput")
    wk_d = nc.dram_tensor("wk", [CTX, HC], BF16, kind="ExternalInput")
    wv_d = nc.dram_tensor("wv", [CTX, HC], BF16, kind="ExternalInput")
    wo_d = nc.dram_tensor("wo", [HC, C], BF16, kind="ExternalInput")
    out_d = nc.dram_tensor("out", [ROWS, C], F32, kind="ExternalOutput")

    xT_t = xT_d.rearrange("(t p) n -> t p n", p=128)     # [8,128,4096]
    cT_t = cT_d.rearrange("(t p) n -> t p n", p=128)     # [6,128,4096]

    with tile.TileContext(nc) as tc:
        with (
            tc.tile_pool(name="const", bufs=1) as const,
            tc.tile_pool(name="ctx_res", bufs=1) as ctx_res,
            tc.tile_pool(name="act_res", bufs=1) as act_res,
            tc.tile_pool(name="xstream", bufs=4) as xstream,
            tc.tile_pool(name="expp", bufs=4) as expp,
            tc.tile_pool(name="otcomb", bufs=2) as otcomb_p,
            tc.tile_pool(name="nrm", bufs=4) as nrm,
            tc.tile_pool(name="ostage", bufs=3) as ostage,
            tc.tile_pool(name="pst", bufs=2, space="PSUM") as pst,
            tc.tile_pool(name="pot", bufs=2, space="PSUM") as pot,
            tc.tile_pool(name="pmm", bufs=2, space="PSUM") as pmm,
        ):
            # ---- weights ----
            wq_sb = const.tile([128, KQ, HC], BF16, tag="wq")
            nc.sync.dma_start(out=wq_sb, in_=wq_d.rearrange("(t p) m -> p t m", p=128))
            wk_sb = const.tile([128, KC, HC], BF16, tag="wk")
            nc.sync.dma_start(out=wk_sb, in_=wk_d.rearrange("(t p) m -> p t m", p=128))
            wv_sb = const.tile([128, KC, HC], BF16, tag="wv")
            nc.sync.dma_start(out=wv_sb, in_=wv_d.rearrange("(t p) m -> p t m", p=128))
            wo_sb = const.tile([128, C], BF16, tag="wo")
            nc.sync.dma_start(out=wo_sb, in_=wo_d[:])

            # ---- resident context (used by kT and v passes) ----
            ctx_sb = ctx_res.tile([128, KC, ROWS], BF16, tag="ctx")
            for t in range(KC):
                nc.sync.dma_start(out=ctx_sb[:, t, :], in_=cT_t[t])

            # ---- kT = Wk^T @ ctxT ----
            kT_sb = act_res.tile([128, ROWS], BF16, tag="kT")
            for n in range(ROWS // NT):
                ps = pmm.tile([128, NT], F32, tag="mmp")
                for t in range(KC):
                    nc.tensor.matmul(ps, wk_sb[:, t, :], ctx_sb[:, t, bass.ts(n, NT)],
                                     start=(t == 0), stop=(t == KC - 1))
                nc.scalar.copy(kT_sb[:, bass.ts(n, NT)], ps)

            # ---- v_aug[m-tile] = [vA | 1 | vB | 1] (130 cols per m-tile) ----
            vago = act_res.tile([128, ROWS // 128, 130], BF16, tag="vaug")
            nc.vector.memset(vago[:, :, 64], 1.0)
            nc.vector.memset(vago[:, :, 129], 1.0)
            for m in range(ROWS // 128):
                ps = pmm.tile([128, NT], F32, tag="mmp")
                for t in range(KC):
                    nc.tensor.matmul(ps[:, 0:HC], ctx_sb[:, t, bass.ts(m, 128)],
                                     wv_sb[:, t, :],
                                     start=(t == 0), stop=(t == KC - 1))
                nc.vector.tensor_copy(vago[:, m, 0:64], ps[:, 0:64])
                nc.vector.tensor_copy(vago[:, m, 65:129], ps[:, 64:128])

            # ---- qT = Wq^T @ xT ----
            qT_sb = act_res.tile([128, ROWS], BF16, tag="qT")
            for n in range(ROWS // NT):
                ps = pmm.tile([128, NT], F32, tag="mmp")
                for t in range(KQ):
                    xs = xstream.tile([128, NT], BF16, tag="xs")
                    nc.sync.dma_start(out=xs, in_=xT_t[t, :, bass.ts(n, NT)])
                    nc.tensor.matmul(ps, wq_sb[:, t, :], xs,
                                     start=(t == 0), stop=(t == KQ - 1))
                nc.scalar.copy(qT_sb[:, bass.ts(n, NT)], ps)

            # ---- attention + output projection ----
            for b in range(B):
                for nt in range(N_NT):
                    nsl = bass.ds(b * N + nt * NT, NT)   # query slice in [0,4096)
                    otc = otcomb_p.tile([128, NT], BF16, tag="otc")
                    for h in range(HEADS_PER_CORE):
                        hd = bass.ds(h * DH, DH)          # head dim rows of qT/kT
                        vsl = bass.ds(h * 65, 65)         # v_aug cols for this head
                        ot_ps = pot.tile([65, NT], F32, tag="ot")
                        for g in range(N_MT // MG):
                            st_ps = pst.tile([128, MG * NT], F32, tag="st")
                            exp_sb = expp.tile([128, MG * NT], BF16, tag="exp")
                            for j in range(MG):
                                mt = g * MG + j
                                msl = bass.ds(b * M + mt * MT, MT)
                                nc.tensor.matmul(
                                    st_ps[:, bass.ts(j, NT)],
                                    kT_sb[hd, msl], qT_sb[hd, nsl],
                                    start=True, stop=True)
                            nc.scalar.activation(
                                exp_sb, st_ps,
                                mybir.ActivationFunctionType.Exp, scale=SCALE)
                            for j in range(MG):
                                mt = g * MG + j
                                nc.tensor.matmul(
                                    ot_ps,
                                    vago[:, (b * M) // 128 + mt, vsl],
                                    exp_sb[:, bass.ts(j, NT)],
                                    start=(mt == 0), stop=(mt == N_MT - 1))
                        # normalize: rows 0:64 divided by denominator row 64
                        rec = nrm.tile([1, NT], F32, tag="rec")
                        nc.vector.reciprocal(rec, ot_ps[64:65, :])
                        bc = nrm.tile([64, NT], F32, tag="bc")
                        nc.gpsimd.partition_broadcast(bc, rec)
                        if h == 0:
                            nc.vector.tensor_mul(otc[0:64, :], ot_ps[0:64, :], bc)
                        else:
                            otn = nrm.tile([64, NT], BF16, tag="otn")
                            nc.vector.tensor_mul(otn, ot_ps[0:64, :], bc)
                            # partition shift 0:64 -> 64:128 (DMA can cross partitions)
                            nc.sync.dma_start(out=otc[64:128, :], in_=otn)
                    # final projection for this query tile
                    for s in range(NT // 128):
                        for cpart in range(C // NT):
                            fp = pmm.tile([128, NT], F32, tag="mmp")
                            nc.tensor.matmul(
                                fp,
                                otc[:, bass.ts(s, 128)],
                                wo_sb[:, bass.ts(cpart, NT)],
                                start=True, stop=True)
                            ost = ostage.tile([128, NT], F32, tag="ost")
                            nc.vector.tensor_copy(ost, fp)
                            nc.sync.dma_start(
                                out=out_d[bass.ds(b * N + nt * NT + s * 128, 128),
                                          bass.ts(cpart, NT)],
                                in_=ost)

    nc.compile()
    return nc


def _shard_inputs(x, context, Wq, Wkv, Wo):
    xf = np.ascontiguousarray(x.reshape(ROWS, C).T).astype(NPBF16)
    cf = np.ascontiguousarray(context.reshape(ROWS, CTX).T).astype(NPBF16)
    in_maps = []
    for c in range(NCORES):
        hc = slice(HC * c, HC * (c + 1))
        in_maps.append({
            "xT": xf,
            "ctxT": cf,
            "wq": np.ascontiguousarray(Wq[:, hc]).astype(NPBF16),
            "wk": np.ascontiguousarray(Wkv[:, hc]).astype(NPBF16),
            "wv": np.ascontiguousarray(Wkv[:, C + HC * c:C + HC * (c + 1)]).astype(NPBF16),
            "wo": np.ascontiguousarray(Wo[hc, :]).astype(NPBF16),
        })
    return in_maps


def get_nc():
    if "nc" not in _CACHE:
        _CACHE["nc"] = build_kernel()
    return _CACHE["nc"]


def run_cores(in_maps, **kw):
    nc = get_nc()
    return run_bass_kernel_spmd(nc, in_maps, list(range(NCORES)), **kw)


def kernel(x, context, Wq, Wkv, Wo):
    in_maps = _shard_inputs(
        np.asarray(x, np.float32), np.asarray(context, np.float32),
        np.asarray(Wq, np.float32), np.asarray(Wkv, np.float32),
        np.asarray(Wo, np.float32))
    res = run_cores(in_maps)
    acc = res.results[0]["out"].astype(np.float32)
    for i in range(1, NCORES):
        acc = acc + res.results[i]["out"]
    return acc.reshape(B, N, C)
